# revision 1
# baseline (speedup 1.0000x reference)
"""PiLoraLayer TRN2 kernel: y = x + (alpha/r) * sin((2/pi) * (x @ A) @ B).

x: [4, 4096, 4096] f32; A = A_int8 * scale_A (per-col), B = B_int8 * scale_B
(per-col); rank 16 bottleneck.

Strategy (data-parallel over 8 NeuronCores):
- Host: dequantize the tiny weights once. Fold scale_A, scale_B and 1/pi^2
  into Bp = scale_A[:,None] * B_q * scale_B[None,:] / pi^2; keep A_q as f32.
  Then u = (x @ A_q) @ Bp equals arg/(2*pi) where arg = (2/pi)*h2, and
  y = x + 2*sin(2*pi*u).
- Shard x's 16384 token rows into 8 x [2048, 4096] shards, one per core.
- Device (per core), per 512-token super-tile:
    - DMA x in (4 chunks of [128, 4096]).
    - PE-transpose x into [128h, 512t] slabs; ACT copies PSUM->SBUF.
    - mm1: h1T[16, 512] = sum_k A_k.T @ xT_k (PSUM accumulate, 32 chunks)
    - mm2: per 128-token chunk, u_psum[128, 1024] = h1_c @ Bp_n (2-bank tile)
    - Range reduction (HW Sin LUT only accepts [-pi, pi]):
      k = (u + 1.5*2^23) - 1.5*2^23 in ONE two-op DVE tensor_scalar (RNE
      round-to-integer, written as bf16 which is exact for |k| <= 256);
      PE accumulates -k into the u bank via a bf16 negative-identity matmul,
      leaving frac in [-0.5, 0.5]; ACT computes s = sin(2*pi*frac) -> bf16.
    - DVE: s *= 2 (bf16 4x mode, in place), x_sb += s (mixed f32+bf16),
      DMA x_sb out as y.
- GPSIMD is kept out of the steady-state loop entirely: it is ~10x slower
  than DVE for elementwise work and its SBUF port sharing starves DVE.
"""

import sys

sys.path.insert(0, "/opt/trn_rl_repo")

import numpy as np

import concourse.bacc as bacc
import concourse.bass as bass
import concourse.tile as tile
from concourse import mybir
from concourse.bass import ts
from concourse.bass_utils import run_bass_kernel_spmd

P = 128
HIDDEN = 4096
RANK = 16
N_CORES = 8
TOTAL_ROWS = 4 * 4096
ROWS = TOTAL_ROWS // N_CORES  # 2048 per core
SUPER = 512  # tokens per steady-state super-tile
NCH = SUPER // P  # token chunks per super-tile
KC = HIDDEN // P  # 32 hidden chunks
UBLK = 1024  # tail block width (2 PSUM banks)
NUB = HIDDEN // UBLK  # 4 tail blocks per token chunk
ALPHA_OVER_R = 2.0  # 32.0 / 16
MAGIC = 12582912.0  # 1.5 * 2^23: f32 add/sub rounds to nearest integer
SCALE_2PI = 6.283185  # slightly < 2*pi so the LUT arg stays inside [-pi, pi]

F32 = mybir.dt.float32
F32R = mybir.dt.float32r  # replicated fp32: 1 cycle/row on PE when N>=256
BF16 = mybir.dt.bfloat16


def build_nc(rows: int = ROWS):
    """Build the per-core Bass program for a [rows, 4096] token shard."""
    assert rows % SUPER == 0
    n_super = rows // SUPER

    nc = bacc.Bacc(
        "TRN2",
        target_bir_lowering=False,
        debug=False,
        enable_asserts=False,
        num_devices=N_CORES,
    )
    x_d = nc.dram_tensor("x", [rows, HIDDEN], F32, kind="ExternalInput").ap()
    a_d = nc.dram_tensor("A", [HIDDEN, RANK], F32, kind="ExternalInput").ap()
    bp_d = nc.dram_tensor("Bp", [RANK, HIDDEN], F32, kind="ExternalInput").ap()
    i_d = nc.dram_tensor("I", [P, P], F32, kind="ExternalInput").ap()
    y_d = nc.dram_tensor("out", [rows, HIDDEN], F32, kind="ExternalOutput").ap()

    with tile.TileContext(nc) as tc:
        with (
            tc.tile_pool(name="singles", bufs=1) as singles,
            tc.tile_pool(name="xp", bufs=2) as xpool,
            tc.tile_pool(name="xtp", bufs=6) as xtpool,
            tc.tile_pool(name="kp", bufs=4) as kpool,
            tc.tile_pool(name="sp", bufs=4) as spool,
            tc.tile_pool(name="h1sb", bufs=2) as h1pool,
            tc.tile_pool(name="ptp", bufs=2, space="PSUM") as pt_psum,
            tc.tile_pool(name="h1p", bufs=2, space="PSUM") as h1_psum,
            tc.tile_pool(name="up", bufs=2, space="PSUM") as u_psum,
        ):
            ident = singles.tile([P, P], F32R)
            nc.sync.dma_start(out=ident[:], in_=i_d[:, :].bitcast(F32R))
            nident_bf = singles.tile([P, P], BF16)
            nc.gpsimd.memset(nident_bf[:], 0.0)
            nc.gpsimd.affine_select(
                out=nident_bf[:],
                in_=nident_bf[:],
                compare_op=mybir.AluOpType.not_equal,
                fill=-1.0,
                base=0,
                pattern=[[-1, P]],
                channel_multiplier=1,
            )
            a_sb = singles.tile([P, KC, RANK], F32R)
            nc.sync.dma_start(
                out=a_sb[:],
                in_=a_d.rearrange("(k p) r -> p k r", p=P).bitcast(F32R),
            )
            bp_sb = singles.tile([RANK, HIDDEN], F32R)
            nc.sync.dma_start(out=bp_sb[:], in_=bp_d[:, :].bitcast(F32R))

            def emit_tail_block(state, j):
                """One 1024-wide tail block j for a finished super-tile."""
                x_sb, h1_sb, row0, _nch = state
                c, nb = j // NUB, j % NUB
                u_ps = u_psum.tile([P, UBLK], F32)
                for jj in range(2):
                    nc.tensor.matmul(
                        u_ps[:, ts(jj, 512)],
                        h1_sb[:, ts(c, P)],
                        bp_sb[:, nb * UBLK + jj * 512 : nb * UBLK + (jj + 1) * 512],
                        start=True,
                        stop=True,
                    )
                kq = kpool.tile([P, UBLK], BF16)
                nc.vector.tensor_scalar(
                    kq[:],
                    u_ps[:],
                    MAGIC,
                    MAGIC,
                    mybir.AluOpType.add,
                    mybir.AluOpType.subtract,
                )
                for jj in range(2):
                    nc.tensor.matmul(
                        u_ps[:, ts(jj, 512)],
                        nident_bf[:],
                        kq[:, ts(jj, 512)],
                        start=False,
                        stop=True,
                        skip_group_check=True,
                    )
                s = spool.tile([P, UBLK], BF16)
                nc.scalar.activation(
                    out=s[:],
                    in_=u_ps[:],
                    func=mybir.ActivationFunctionType.Sin,
                    scale=SCALE_2PI,
                )
                nc.vector.tensor_scalar_mul(s[:], s[:], ALPHA_OVER_R)
                nc.vector.tensor_tensor(
                    x_sb[:, c, nb * UBLK : (nb + 1) * UBLK].bitcast(F32R),
                    x_sb[:, c, nb * UBLK : (nb + 1) * UBLK],
                    s[:],
                    mybir.AluOpType.add,
                )
                if nb == NUB - 1:
                    r0 = row0 + c * P
                    nc.gpsimd.dma_start(out=y_d[r0 : r0 + P, :], in_=x_sb[:, c, :])

            # super-tile layout: small first/last tiles halve pipeline
            # fill (k-loop with no tail to hide) and drain (tail with no
            # k-loop to hide)
            layout = []
            r = 0
            sizes = [256] + [SUPER] * ((rows - 512) // SUPER) + [256]
            if rows <= 512:
                sizes = [rows]
            for tok in sizes:
                layout.append((r, tok))
                r += tok
            assert r == rows

            prev = None  # (x_sb, h1_sb, row0, nch) of the previous super-tile

            for st, (row0, tok) in enumerate(layout):
                nch = tok // P
                x_sb = xpool.tile([P, nch, HIDDEN], F32)
                # column-half loads (kb-major) so the first transposes can
                # start after ~1/2 of the super-tile's data has landed
                for kb in range(2):
                    cols = slice(kb * (HIDDEN // 2), (kb + 1) * (HIDDEN // 2))
                    for c in range(nch):
                        r0 = row0 + c * P
                        nc.sync.dma_start(
                            out=x_sb[:, c, cols].bitcast(F32R),
                            in_=x_d[r0 : r0 + P, cols].bitcast(F32R),
                        )

                # mm1 k-loop of st, interleaved with the tail blocks of st-1
                ntail_prev = prev[3] * NUB if prev is not None else 0
                stride = KC // ntail_prev if ntail_prev else 0
                h1_ps = h1_psum.tile([RANK, tok], F32)
                for k in range(KC):
                    pt = pt_psum.tile([P, nch, P], F32R)
                    for c in range(nch):
                        nc.tensor.transpose(
                            pt[:, c, :],
                            x_sb[:, c, ts(k, P)].bitcast(F32R),
                            ident[:],
                        )
                    xt = xtpool.tile([P, tok], F32R)
                    nc.scalar.copy(out=xt[:], in_=pt[:])
                    nc.tensor.matmul(
                        h1_ps[:],
                        a_sb[:, k, :],
                        xt[:],
                        start=(k == 0),
                        stop=(k == KC - 1),
                    )
                    if ntail_prev and k % stride == stride - 1:
                        emit_tail_block(prev, k // stride)
                h1_sb = h1pool.tile([RANK, tok], F32R)
                nc.vector.tensor_copy(h1_sb[:], h1_ps[:])
                prev = (x_sb, h1_sb, row0, nch)

            # drain: the last super-tile's tail has no successor to hide in
            for j in range(prev[3] * NUB):
                emit_tail_block(prev, j)

    nc.compile()
    return nc


_NC_CACHE: dict[int, object] = {}


def _get_nc(rows: int = ROWS):
    nc = _NC_CACHE.get(rows)
    if nc is None:
        nc = build_nc(rows)
        _NC_CACHE[rows] = nc
    return nc


def _prep_weights(A_int8, B_int8, scale_A, scale_B):
    a_f = np.ascontiguousarray(A_int8.astype(np.float32))
    bp = np.ascontiguousarray(
        scale_A.astype(np.float32)[:, None]
        * B_int8.astype(np.float32)
        * scale_B.astype(np.float32)[None, :]
        * np.float32(1.0 / (np.pi * np.pi))
    )
    return a_f, bp


def kernel(x, A_int8, B_int8, scale_A, scale_B):
    x = np.asarray(x)
    orig_shape = x.shape
    xf = np.ascontiguousarray(x.reshape(TOTAL_ROWS, HIDDEN).astype(np.float32))
    a_f, bp = _prep_weights(
        np.asarray(A_int8), np.asarray(B_int8), np.asarray(scale_A), np.asarray(scale_B)
    )

    nc = _get_nc(ROWS)
    eye = np.eye(P, dtype=np.float32)
    in_maps = [
        {"x": xf[i * ROWS : (i + 1) * ROWS], "A": a_f, "Bp": bp, "I": eye}
        for i in range(N_CORES)
    ]
    res = run_bass_kernel_spmd(nc, in_maps, core_ids=list(range(N_CORES)))
    y = np.concatenate([r["out"] for r in res.results], axis=0)
    return y.reshape(orig_shape).astype(np.float32)



# revision 17
# speedup vs baseline: 1.1668x; 1.1668x over previous
"""PiLoraLayer TRN2 kernel: y = x + (alpha/r) * sin((2/pi) * (x @ A) @ B).

x: [4, 4096, 4096] f32; A = A_int8 * scale_A (per-col), B = B_int8 * scale_B
(per-col); rank 16 bottleneck.

Strategy (data-parallel over 8 NeuronCores, fully transposed dataflow):
- Host: xh = (x/2) as fp16 (residual+matmul source), A2 = 2*A_int8 as fp16,
  Bp = scale_A[:,None] * B_q * scale_B[None,:] / pi^2 (f32). Then
  u = (xh @ A2) @ Bp = arg/(2*pi) and y = 2*(xh + sin(2*pi*frac(u))).
- Shard x's 16384 token rows into 8 x [2048, 4096] shards, one per core.
- Device (per core), per super-tile of T tokens, ALL in transposed space:
    - ONE dma_start_transpose loads xT slab [128h, KC, T] (xbar HW transpose)
    - mm1: h1T[16, T] = sum_k A2_k.T @ xT_k (PSUM accumulate, 32 chunks)
    - mm2 (transposed): uT chunk [128h, T] = Bp_chunk.T @ h1T -- Bp slices
      are natural stationaries, no transpose of anything needed (f32r for
      precision; fp16 Bp would add ~5e-3 rel err).
    - Range reduction per 1024-elem block: kq = (u + 1.5*2^23) - 1.5*2^23
      (one DVE tensor_scalar, RNE-to-integer, fp16 exact for |k|<=2048);
      PE accumulates -kq via fp16 negative-identity matmul -> frac in PSUM;
      ACT: s = sin(2*pi*frac) -> fp16.
    - DVE fp16 TT add (2x mode): slab += s  (slab holds xh.T = x.T/2)
    - DMA slab chunk out as y.T/2; host returns 2 * y_t.T.
- No PE transposes (HAM-blind + PSUM copies), no ACT copies, single x read,
  fp16 I/O: 3x less DMA bytes and ~2x less engine work than the v1 kernel.
"""

import os
import sys

sys.path.insert(0, "/opt/trn_rl_repo")

import numpy as np

HOST_T = bool(os.environ.get("HOST_T"))  # debug: host-side x transpose
DBG = os.environ.get("DBG", "")  # debug: "echo" = skip add, "sin" = only sin part
YW = os.environ.get("YW", "gpsimd")  # y-write engine: gpsimd (SWDGE) | act (HWDGE)

import concourse.bacc as bacc
import concourse.tile as tile
from concourse import mybir
from concourse.bass import ts
from concourse.bass_utils import run_bass_kernel_spmd

P = 128
HIDDEN = 4096
RANK = 16
N_CORES = 8
TOTAL_ROWS = 4 * 4096
ROWS = TOTAL_ROWS // N_CORES  # 2048 per core
SUPER = 512  # tokens per steady-state super-tile
KC = HIDDEN // P  # 32 hidden chunks
BLK = 1024  # tail block free elems (2 PSUM banks)
MAGIC = 12582912.0  # 1.5 * 2^23: f32 add/sub rounds to nearest integer
SCALE_2PI = 6.283185  # slightly < 2*pi so the LUT arg stays inside [-pi, pi]

F32 = mybir.dt.float32
F32R = mybir.dt.float32r
F16 = mybir.dt.float16


def build_nc(rows: int = ROWS):
    """Build the per-core Bass program for a [rows, 4096] token shard."""
    nc = bacc.Bacc(
        "TRN2",
        target_bir_lowering=False,
        debug=False,
        enable_asserts=False,
        num_devices=N_CORES,
    )
    if HOST_T:
        x_d = nc.dram_tensor("x", [HIDDEN, rows], F16, kind="ExternalInput").ap()
        x_t = x_d.rearrange("(k p) t -> p k t", p=P)
    else:
        x_d = nc.dram_tensor("x", [rows, HIDDEN], F16, kind="ExternalInput").ap()
    a_d = nc.dram_tensor("A", [HIDDEN, RANK], F16, kind="ExternalInput").ap()
    bp_d = nc.dram_tensor("Bp", [RANK, HIDDEN], F32, kind="ExternalInput").ap()
    y_d = nc.dram_tensor("out", [HIDDEN, rows], F16, kind="ExternalOutput").ap()
    y_r = y_d.rearrange("(k p) t -> p k t", p=P)  # [128, KC, rows]

    # uniform super-tiles: PSUM matmul outputs must stay bank-aligned, which
    # holds only for T=512 (chunk writes at 0/512 f32 offsets)
    assert rows % SUPER == 0
    layout = [(i * SUPER, SUPER) for i in range(rows // SUPER)]

    with tile.TileContext(nc) as tc:
        with (
            tc.tile_pool(name="singles", bufs=1) as singles,
            tc.tile_pool(name="slabp", bufs=3) as slab_pool,
            tc.tile_pool(name="kp", bufs=4) as kpool,
            tc.tile_pool(name="sp", bufs=4) as spool,
            tc.tile_pool(name="h1sb", bufs=2) as h1pool,
            tc.tile_pool(name="h1p", bufs=2, space="PSUM") as h1_psum,
            tc.tile_pool(name="up", bufs=2, space="PSUM") as u_psum,
        ):
            nident = singles.tile([P, P], F16)
            nc.gpsimd.memset(nident[:], 0.0)
            nc.gpsimd.affine_select(
                out=nident[:],
                in_=nident[:],
                compare_op=mybir.AluOpType.not_equal,
                fill=-1.0,
                base=0,
                pattern=[[-1, P]],
                channel_multiplier=1,
            )
            a_sb = singles.tile([P, KC, RANK], F16)
            nc.sync.dma_start(
                out=a_sb[:], in_=a_d.rearrange("(k p) r -> p k r", p=P)
            )
            # NOTE: do NOT dma with .bitcast(F32R) in a program that also uses
            # dma_start_transpose -- the f32r-tagged DMA descriptor poisons the
            # xbar path and the transposed f16 data comes back f32r-rounded
            # (even 16-bit lanes quantized). Load plain f32, then produce the
            # f32r copy on-chip with DVE (a legal f32r producer for the PE).
            bp_f32 = singles.tile([RANK, HIDDEN], F32)
            nc.sync.dma_start(out=bp_f32[:], in_=bp_d[:, :])
            bp_sb = singles.tile([RANK, HIDDEN], F32R)
            nc.vector.tensor_copy(bp_sb[:], bp_f32[:])

            def emit_tail(state):
                """Tail for a finished super-tile: mm2T + sin + residual."""
                slab, h1_sb, t0, T = state
                cb = BLK // T  # hidden chunks per block
                nb = KC // cb  # blocks per super-tile
                ydma = nc.scalar.dma_start if YW == "act" else nc.gpsimd.dma_start
                if DBG in ("pure", "mm1"):
                    for n in range(nb):
                        ydma(
                            out=y_r[:, n * cb : (n + 1) * cb, t0 : t0 + T],
                            in_=slab[:, n * cb : (n + 1) * cb, :],
                        )
                    return
                for n in range(nb):
                    u_ps = u_psum.tile([P, cb, T], F32)
                    for c in range(cb):
                        nc.tensor.matmul(
                            u_ps[:, c, :],
                            bp_sb[:, ts(n * cb + c, P)],
                            h1_sb[:],
                            start=True,
                            stop=True,
                        )
                    kq = kpool.tile([P, cb, T], F16)
                    nc.vector.tensor_scalar(
                        kq[:],
                        u_ps[:],
                        MAGIC,
                        MAGIC,
                        mybir.AluOpType.add,
                        mybir.AluOpType.subtract,
                    )
                    for c in range(cb):
                        nc.tensor.matmul(
                            u_ps[:, c, :],
                            nident[:],
                            kq[:, c, :],
                            start=False,
                            stop=True,
                            skip_group_check=True,
                        )
                    s = spool.tile([P, cb, T], F16)
                    nc.scalar.activation(
                        out=s[:],
                        in_=u_ps[:],
                        func=mybir.ActivationFunctionType.Sin,
                        scale=SCALE_2PI,
                    )
                    if DBG == "echo":
                        pass
                    elif DBG == "sin":
                        nc.vector.tensor_copy(slab[:, n * cb : (n + 1) * cb, :], s[:])
                    else:
                        nc.vector.tensor_tensor(
                            slab[:, n * cb : (n + 1) * cb, :],
                            slab[:, n * cb : (n + 1) * cb, :],
                            s[:],
                            mybir.AluOpType.add,
                        )
                    ydma(
                        out=y_r[:, n * cb : (n + 1) * cb, t0 : t0 + T],
                        in_=slab[:, n * cb : (n + 1) * cb, :],
                    )

            prev = None
            for st, (t0, T) in enumerate(layout):
                slab = slab_pool.tile([P, KC, T], F16)
                if HOST_T:
                    nc.sync.dma_start(out=slab[:], in_=x_t[:, :, t0 : t0 + T])
                else:
                    nc.sync.dma_start_transpose(out=slab[:], in_=x_d[t0 : t0 + T, :])
                if prev is not None:
                    emit_tail(prev)
                h1_sb = None
                if DBG != "pure":
                    h1_ps = h1_psum.tile([RANK, T], F32)
                    for k in range(KC):
                        nc.tensor.matmul(
                            h1_ps[:],
                            a_sb[:, k, :],
                            slab[:, k, :],
                            start=(k == 0),
                            stop=(k == KC - 1),
                        )
                    h1_sb = h1pool.tile([RANK, T], F32R)
                    nc.vector.tensor_copy(h1_sb[:], h1_ps[:])
                prev = (slab, h1_sb, t0, T)

            emit_tail(prev)

    nc.compile()
    return nc


_NC_CACHE: dict[int, object] = {}


def _get_nc(rows: int = ROWS):
    nc = _NC_CACHE.get(rows)
    if nc is None:
        nc = build_nc(rows)
        _NC_CACHE[rows] = nc
    return nc


def _prep_weights(A_int8, B_int8, scale_A, scale_B):
    a2 = np.ascontiguousarray((A_int8.astype(np.float32) * 2.0).astype(np.float16))
    bp = np.ascontiguousarray(
        scale_A.astype(np.float32)[:, None]
        * B_int8.astype(np.float32)
        * scale_B.astype(np.float32)[None, :]
        * np.float32(1.0 / (np.pi * np.pi))
    )
    return a2, bp


def _prep_in_maps(x, A_int8, B_int8, scale_A, scale_B, rows=ROWS, n_cores=N_CORES):
    xf = x.reshape(-1, HIDDEN)
    a2, bp = _prep_weights(A_int8, B_int8, scale_A, scale_B)
    def shard(i):
        xh = (xf[i * rows : (i + 1) * rows] * np.float32(0.5)).astype(np.float16)
        if HOST_T:
            xh = xh.T
        return np.ascontiguousarray(xh)

    return [
        {"x": shard(i), "A": a2, "Bp": bp}
        for i in range(n_cores)
    ]


def _postprocess(results, orig_shape):
    parts = [
        (np.ascontiguousarray(r["out"].T).astype(np.float32) * np.float32(2.0))
        for r in results
    ]
    return np.concatenate(parts, axis=0).reshape(orig_shape)


def kernel(x, A_int8, B_int8, scale_A, scale_B):
    x = np.asarray(x)
    orig_shape = x.shape
    in_maps = _prep_in_maps(
        np.ascontiguousarray(x.reshape(TOTAL_ROWS, HIDDEN)),
        np.asarray(A_int8),
        np.asarray(B_int8),
        np.asarray(scale_A),
        np.asarray(scale_B),
    )
    nc = _get_nc(ROWS)
    res = run_bass_kernel_spmd(nc, in_maps, core_ids=list(range(N_CORES)))
    return _postprocess(res.results, orig_shape)


# revision 21
# speedup vs baseline: 1.2460x; 1.0679x over previous
"""PiLoraLayer TRN2 kernel: y = x + (alpha/r) * sin((2/pi) * (x @ A) @ B).

x: [4, 4096, 4096] f32; A = A_int8 * scale_A (per-col), B = B_int8 * scale_B
(per-col); rank 16 bottleneck.

Strategy (data-parallel over 8 NeuronCores, fully transposed dataflow):
- Host: xh = (x/2) as fp16 (residual+matmul source), A2 = 2*A_int8 as fp16,
  Bp = scale_A[:,None] * B_q * scale_B[None,:] / pi^2 (f32). Then
  u = (xh @ A2) @ Bp = arg/(2*pi) and y = 2*(xh + sin(2*pi*frac(u))).
- Shard x's 16384 token rows into 8 x [2048, 4096] shards, one per core.
- Device (per core), per super-tile of T tokens, ALL in transposed space:
    - ONE dma_start_transpose loads xT slab [128h, KC, T] (xbar HW transpose)
    - mm1: h1T[16, T] = sum_k A2_k.T @ xT_k (PSUM accumulate, 32 chunks)
    - mm2 (transposed): uT chunk [128h, T] = Bp_chunk.T @ h1T -- Bp slices
      are natural stationaries, no transpose of anything needed (f32r for
      precision; fp16 Bp would add ~5e-3 rel err).
    - Range reduction per 1024-elem block: kq = (u + 1.5*2^23) - 1.5*2^23
      (one DVE tensor_scalar, RNE-to-integer, fp16 exact for |k|<=2048);
      PE accumulates -kq via fp16 negative-identity matmul -> frac in PSUM;
      ACT: s = sin(2*pi*frac) -> fp16.
    - DVE fp16 TT add (2x mode): slab += s  (slab holds xh.T = x.T/2)
    - DMA slab chunk out as y.T/2; host returns 2 * y_t.T.
- No PE transposes (HAM-blind + PSUM copies), no ACT copies, single x read,
  fp16 I/O: 3x less DMA bytes and ~2x less engine work than the v1 kernel.
"""

import os
import sys

sys.path.insert(0, "/opt/trn_rl_repo")

import numpy as np

HOST_T = bool(os.environ.get("HOST_T"))  # debug: host-side x transpose
DBG = os.environ.get("DBG", "")  # debug: "echo" = skip add, "sin" = only sin part
YW = os.environ.get("YW", "gpsimd")  # y-write engine: gpsimd (SWDGE) | act (HWDGE)
ROUND_ENG = os.environ.get("ROUND", "vector")  # round engine (GPSIMD can't read PSUM)

import concourse.bacc as bacc
import concourse.tile as tile
from concourse import mybir
from concourse.bass import ts
from concourse.bass_utils import run_bass_kernel_spmd

P = 128
HIDDEN = 4096
RANK = 16
N_CORES = 8
TOTAL_ROWS = 4 * 4096
ROWS = TOTAL_ROWS // N_CORES  # 2048 per core
SUPER = 512  # tokens per steady-state super-tile
KC = HIDDEN // P  # 32 hidden chunks
BLK = 1024  # tail block free elems (2 PSUM banks)
MAGIC = 12582912.0  # 1.5 * 2^23: f32 add/sub rounds to nearest integer
SCALE_2PI = 6.283185  # slightly < 2*pi so the LUT arg stays inside [-pi, pi]

F32 = mybir.dt.float32
F32R = mybir.dt.float32r
F16 = mybir.dt.float16


def build_nc(rows: int = ROWS):
    """Build the per-core Bass program for a [rows, 4096] token shard."""
    nc = bacc.Bacc(
        "TRN2",
        target_bir_lowering=False,
        debug=False,
        enable_asserts=False,
        num_devices=N_CORES,
    )
    if HOST_T:
        x_d = nc.dram_tensor("x", [HIDDEN, rows], F16, kind="ExternalInput").ap()
        x_t = x_d.rearrange("(k p) t -> p k t", p=P)
    else:
        x_d = nc.dram_tensor("x", [rows, HIDDEN], F16, kind="ExternalInput").ap()
    a_d = nc.dram_tensor("A", [HIDDEN, RANK], F16, kind="ExternalInput").ap()
    bp_d = nc.dram_tensor("Bp", [RANK, HIDDEN], F32, kind="ExternalInput").ap()
    y_d = nc.dram_tensor("out", [HIDDEN, rows], F16, kind="ExternalOutput").ap()
    y_r = y_d.rearrange("(k p) t -> p k t", p=P)  # [128, KC, rows]

    # uniform super-tiles: PSUM matmul outputs must stay bank-aligned, which
    # holds only for T=512 (chunk writes at 0/512 f32 offsets)
    assert rows % SUPER == 0
    layout = [(i * SUPER, SUPER) for i in range(rows // SUPER)]

    with tile.TileContext(nc) as tc:
        with (
            tc.tile_pool(name="singles", bufs=1) as singles,
            tc.tile_pool(name="slabp", bufs=3) as slab_pool,
            tc.tile_pool(name="kp", bufs=4) as kpool,
            tc.tile_pool(name="sp", bufs=4) as spool,
            tc.tile_pool(name="h1sb", bufs=2) as h1pool,
            tc.tile_pool(name="h1p", bufs=2, space="PSUM") as h1_psum,
            tc.tile_pool(name="up", bufs=3, space="PSUM") as u_psum,
        ):
            nident = singles.tile([P, P], F16)
            nc.gpsimd.memset(nident[:], 0.0)
            nc.gpsimd.affine_select(
                out=nident[:],
                in_=nident[:],
                compare_op=mybir.AluOpType.not_equal,
                fill=-1.0,
                base=0,
                pattern=[[-1, P]],
                channel_multiplier=1,
            )
            a_sb = singles.tile([P, KC, RANK], F16)
            nc.sync.dma_start(
                out=a_sb[:], in_=a_d.rearrange("(k p) r -> p k r", p=P)
            )
            # NOTE: do NOT dma with .bitcast(F32R) in a program that also uses
            # dma_start_transpose -- the f32r-tagged DMA descriptor poisons the
            # xbar path and the transposed f16 data comes back f32r-rounded
            # (even 16-bit lanes quantized). Load plain f32, then produce the
            # f32r copy on-chip with DVE (a legal f32r producer for the PE).
            bp_f32 = singles.tile([RANK, HIDDEN], F32)
            nc.sync.dma_start(out=bp_f32[:], in_=bp_d[:, :])
            bp_sb = singles.tile([RANK, HIDDEN], F32R)
            nc.vector.tensor_copy(bp_sb[:], bp_f32[:])

            round_eng = nc.gpsimd if ROUND_ENG == "gpsimd" else nc.vector
            WG = 4  # tail blocks per y-write DMA

            def emit_tail(state):
                """Tail for a finished super-tile, software-pipelined:
                FILL runs one block ahead of SUB so the PE never stalls on
                the round; y-writes batch WG blocks and trail the adds."""
                slab, h1_sb, t0, T = state
                cb = BLK // T  # hidden chunks per block
                nb = KC // cb  # blocks per super-tile
                ydma = nc.scalar.dma_start if YW == "act" else nc.gpsimd.dma_start

                def emit_write(g):
                    lo, hi = g * WG * cb, (g + 1) * WG * cb
                    ydma(
                        out=y_r[:, lo:hi, t0 : t0 + T],
                        in_=slab[:, lo:hi, :],
                    )

                if DBG in ("pure", "mm1"):
                    for g in range(nb // WG):
                        emit_write(g)
                    return

                def emit_fill(n):
                    u_ps = u_psum.tile([P, cb, T], F32)
                    for c in range(cb):
                        nc.tensor.matmul(
                            u_ps[:, c, :],
                            bp_sb[:, ts(n * cb + c, P)],
                            h1_sb[:],
                            start=True,
                            stop=True,
                        )
                    return u_ps

                u_tiles = {0: emit_fill(0)}
                written = 0
                for n in range(nb):
                    if n + 1 < nb:
                        u_tiles[n + 1] = emit_fill(n + 1)
                    u_ps = u_tiles.pop(n)
                    kq = kpool.tile([P, cb, T], F16)
                    round_eng.tensor_scalar(
                        kq[:],
                        u_ps[:],
                        MAGIC,
                        MAGIC,
                        mybir.AluOpType.add,
                        mybir.AluOpType.subtract,
                    )
                    for c in range(cb):
                        nc.tensor.matmul(
                            u_ps[:, c, :],
                            nident[:],
                            kq[:, c, :],
                            start=False,
                            stop=True,
                            skip_group_check=True,
                        )
                    s = spool.tile([P, cb, T], F16)
                    nc.scalar.activation(
                        out=s[:],
                        in_=u_ps[:],
                        func=mybir.ActivationFunctionType.Sin,
                        scale=SCALE_2PI,
                    )
                    if DBG == "echo":
                        pass
                    elif DBG == "sin":
                        nc.vector.tensor_copy(slab[:, n * cb : (n + 1) * cb, :], s[:])
                    else:
                        nc.vector.tensor_tensor(
                            slab[:, n * cb : (n + 1) * cb, :],
                            slab[:, n * cb : (n + 1) * cb, :],
                            s[:],
                            mybir.AluOpType.add,
                        )
                    # write group g once its last add is >= WG blocks behind
                    if (n + 1) % WG == 0 and (n + 1) // WG >= 2:
                        emit_write(written)
                        written += 1
                for g in range(written, nb // WG):
                    emit_write(g)

            prev = None
            for st, (t0, T) in enumerate(layout):
                slab = slab_pool.tile([P, KC, T], F16)
                if HOST_T:
                    nc.sync.dma_start(out=slab[:], in_=x_t[:, :, t0 : t0 + T])
                else:
                    nc.sync.dma_start_transpose(out=slab[:], in_=x_d[t0 : t0 + T, :])
                if prev is not None:
                    emit_tail(prev)
                h1_sb = None
                if DBG != "pure":
                    h1_ps = h1_psum.tile([RANK, T], F32)
                    for k in range(KC):
                        nc.tensor.matmul(
                            h1_ps[:],
                            a_sb[:, k, :],
                            slab[:, k, :],
                            start=(k == 0),
                            stop=(k == KC - 1),
                        )
                    h1_sb = h1pool.tile([RANK, T], F32R)
                    nc.vector.tensor_copy(h1_sb[:], h1_ps[:])
                prev = (slab, h1_sb, t0, T)

            emit_tail(prev)

    nc.compile()
    return nc


_NC_CACHE: dict[int, object] = {}


def _get_nc(rows: int = ROWS):
    nc = _NC_CACHE.get(rows)
    if nc is None:
        nc = build_nc(rows)
        _NC_CACHE[rows] = nc
    return nc


def _prep_weights(A_int8, B_int8, scale_A, scale_B):
    a2 = np.ascontiguousarray((A_int8.astype(np.float32) * 2.0).astype(np.float16))
    bp = np.ascontiguousarray(
        scale_A.astype(np.float32)[:, None]
        * B_int8.astype(np.float32)
        * scale_B.astype(np.float32)[None, :]
        * np.float32(1.0 / (np.pi * np.pi))
    )
    return a2, bp


def _prep_in_maps(x, A_int8, B_int8, scale_A, scale_B, rows=ROWS, n_cores=N_CORES):
    xf = x.reshape(-1, HIDDEN)
    a2, bp = _prep_weights(A_int8, B_int8, scale_A, scale_B)
    def shard(i):
        xh = (xf[i * rows : (i + 1) * rows] * np.float32(0.5)).astype(np.float16)
        if HOST_T:
            xh = xh.T
        return np.ascontiguousarray(xh)

    return [
        {"x": shard(i), "A": a2, "Bp": bp}
        for i in range(n_cores)
    ]


def _postprocess(results, orig_shape):
    parts = [
        (np.ascontiguousarray(r["out"].T).astype(np.float32) * np.float32(2.0))
        for r in results
    ]
    return np.concatenate(parts, axis=0).reshape(orig_shape)


def kernel(x, A_int8, B_int8, scale_A, scale_B):
    x = np.asarray(x)
    orig_shape = x.shape
    in_maps = _prep_in_maps(
        np.ascontiguousarray(x.reshape(TOTAL_ROWS, HIDDEN)),
        np.asarray(A_int8),
        np.asarray(B_int8),
        np.asarray(scale_A),
        np.asarray(scale_B),
    )
    nc = _get_nc(ROWS)
    res = run_bass_kernel_spmd(nc, in_maps, core_ids=list(range(N_CORES)))
    return _postprocess(res.results, orig_shape)


# revision 35
# speedup vs baseline: 1.3532x; 1.0861x over previous
"""PiLoraLayer TRN2 kernel: y = x + (alpha/r) * sin((2/pi) * (x @ A) @ B).

x: [4, 4096, 4096] f32; A = A_int8 * scale_A (per-col), B = B_int8 * scale_B
(per-col); rank 16 bottleneck.

Strategy (data-parallel over 8 NeuronCores, fully transposed dataflow):
- Host: xh = (x/2) as fp16 (residual+matmul source), A2 = 2*A_int8 as fp16,
  Bp = scale_A[:,None] * B_q * scale_B[None,:] / pi^2 (f32). Then
  u = (xh @ A2) @ Bp = arg/(2*pi) and y = 2*(xh + sin(2*pi*frac(u))).
- Shard x's 16384 token rows into 8 x [2048, 4096] shards, one per core.
- Device (per core), per super-tile of T tokens, ALL in transposed space:
    - ONE dma_start_transpose loads xT slab [128h, KC, T] (xbar HW transpose)
    - mm1: h1T[16, T] = sum_k A2_k.T @ xT_k (PSUM accumulate, 32 chunks)
    - mm2 (transposed): uT chunk [128h, T] = Bp_chunk.T @ h1T -- Bp slices
      are natural stationaries, no transpose of anything needed (f32r for
      precision; fp16 Bp would add ~5e-3 rel err).
    - Range reduction per 1024-elem block: kq = (u + 1.5*2^23) - 1.5*2^23
      (one DVE tensor_scalar, RNE-to-integer, fp16 exact for |k|<=2048);
      PE accumulates -kq via fp16 negative-identity matmul -> frac in PSUM;
      ACT: s = sin(2*pi*frac) -> fp16.
    - DVE fp16 TT add (2x mode): slab += s  (slab holds xh.T = x.T/2)
    - DMA slab chunk out as y.T/2; host returns 2 * y_t.T.
- No PE transposes (HAM-blind + PSUM copies), no ACT copies, single x read,
  fp16 I/O: 3x less DMA bytes and ~2x less engine work than the v1 kernel.
"""

import os
import sys

sys.path.insert(0, "/opt/trn_rl_repo")

import numpy as np

HOST_T = bool(os.environ.get("HOST_T"))  # debug: host-side x transpose
DBG = os.environ.get("DBG", "")  # debug: "echo" = skip add, "sin" = only sin part
YW = os.environ.get("YW", "gpsimd")  # y-write engine: gpsimd (SWDGE) | act (HWDGE)
ROUND_ENG = os.environ.get("ROUND", "vector")  # round engine (GPSIMD can't read PSUM)
ADD_ENG = os.environ.get("ADD", "vector")  # residual-add engine: vector | gpsimd

import concourse.bacc as bacc
import concourse.tile as tile
from concourse import mybir
from concourse.bass import ts
from concourse.bass_utils import run_bass_kernel_spmd

P = 128
HIDDEN = 4096
RANK = 16
N_CORES = 8
TOTAL_ROWS = 4 * 4096
ROWS = TOTAL_ROWS // N_CORES  # 2048 per core
SUPER = 512  # tokens per steady-state super-tile
KC = HIDDEN // P  # 32 hidden chunks
BLK = 1024  # tail block free elems (2 PSUM banks)
MAGIC = 12582912.0  # 1.5 * 2^23: f32 add/sub rounds to nearest integer
SCALE_2PI = 6.283185  # slightly < 2*pi so the LUT arg stays inside [-pi, pi]

F32 = mybir.dt.float32
F32R = mybir.dt.float32r
F16 = mybir.dt.float16


def build_nc(rows: int = ROWS):
    """Build the per-core Bass program for a [rows, 4096] token shard."""
    nc = bacc.Bacc(
        "TRN2",
        target_bir_lowering=False,
        debug=False,
        enable_asserts=False,
        num_devices=N_CORES,
    )
    if HOST_T:
        x_d = nc.dram_tensor("x", [HIDDEN, rows], F16, kind="ExternalInput").ap()
        x_t = x_d.rearrange("(k p) t -> p k t", p=P)
    else:
        x_d = nc.dram_tensor("x", [rows, HIDDEN], F16, kind="ExternalInput").ap()
    a_d = nc.dram_tensor("A", [HIDDEN, RANK], F16, kind="ExternalInput").ap()
    bp_d = nc.dram_tensor("Bp", [RANK, HIDDEN], F32, kind="ExternalInput").ap()
    rep_d = nc.dram_tensor("REP", [RANK, P], F32, kind="ExternalInput").ap()
    y_d = nc.dram_tensor("out", [HIDDEN, rows], F16, kind="ExternalOutput").ap()
    y_r = y_d.rearrange("(k p) t -> p k t", p=P)  # [128, KC, rows]

    # uniform super-tiles: PSUM matmul outputs must stay bank-aligned, which
    # holds only for T=512 (chunk writes at 0/512 f32 offsets)
    assert rows % SUPER == 0
    layout = [(i * SUPER, SUPER) for i in range(rows // SUPER)]

    with tile.TileContext(nc) as tc:
        with (
            tc.tile_pool(name="singles", bufs=1) as singles,
            tc.tile_pool(name="slabp", bufs=3) as slab_pool,
            tc.tile_pool(name="kp", bufs=4) as kpool,
            tc.tile_pool(name="sp", bufs=4) as spool,
            tc.tile_pool(name="h1sb", bufs=2) as h1pool,
            tc.tile_pool(name="h1p", bufs=1, space="PSUM") as h1_psum,
            tc.tile_pool(name="up", bufs=3, space="PSUM") as u_psum,
        ):
            nident = singles.tile([P, P], F16)
            nc.gpsimd.memset(nident[:], 0.0)
            nc.gpsimd.affine_select(
                out=nident[:],
                in_=nident[:],
                compare_op=mybir.AluOpType.not_equal,
                fill=-1.0,
                base=0,
                pattern=[[-1, P]],
                channel_multiplier=1,
            )
            a_sb = singles.tile([P, KC, RANK], F16)
            nc.sync.dma_start(
                out=a_sb[:], in_=a_d.rearrange("(k p) r -> p k r", p=P)
            )
            # NOTE: do NOT dma with .bitcast(F32R) in a program that also uses
            # dma_start_transpose -- the f32r-tagged DMA descriptor poisons the
            # xbar path and the transposed f16 data comes back f32r-rounded
            # (even 16-bit lanes quantized). Load plain f32, then produce the
            # f32r copy on-chip with DVE (a legal f32r producer for the PE).
            # bp4[32i : 32i+16, g, :] = Bp[:, (4g+i)*128 : (4g+i+1)*128] --
            # the 4 partition groups let 4 rank-16 fill matmuls run
            # concurrently in distinct PE row groups (tile_position).
            bp4_f32 = singles.tile([P, KC // 4, P], F32)
            bp_r4 = bp_d.rearrange("r (g i c) -> r g i c", g=KC // 4, i=4)
            for i in range(4):
                nc.sync.dma_start(
                    out=bp4_f32[32 * i : 32 * i + RANK, :, :],
                    in_=bp_r4[:, :, i, :],
                )
            bp4 = singles.tile([P, KC // 4, P], F32R)
            for i in range(4):
                nc.vector.tensor_copy(
                    bp4[32 * i : 32 * i + RANK, :, :],
                    bp4_f32[32 * i : 32 * i + RANK, :, :],
                )
            # REP[r, 32i+r] = 1: one PE matmul replicates h1T into all four
            # 32-partition groups so packed fills see matching partition bases
            rep_f32 = singles.tile([RANK, P], F32)
            nc.sync.dma_start(out=rep_f32[:], in_=rep_d[:, :])
            rep_sb = singles.tile([RANK, P], F32R)
            nc.vector.tensor_copy(rep_sb[:], rep_f32[:])

            round_eng = nc.gpsimd if ROUND_ENG == "gpsimd" else nc.vector
            WG = 4  # tail blocks per y-write DMA

            def emit_tail(state):
                """Tail for a finished super-tile, software-pipelined:
                FILL runs one block ahead of SUB so the PE never stalls on
                the round; y-writes batch WG blocks and trail the adds."""
                slab, h1_sb, t0, T = state
                cb = BLK // T  # hidden chunks per block
                nb = KC // cb  # blocks per super-tile
                ydma = nc.scalar.dma_start if YW == "act" else nc.gpsimd.dma_start

                def emit_write(g):
                    lo, hi = g * WG * cb, (g + 1) * WG * cb
                    ydma(
                        out=y_r[:, lo:hi, t0 : t0 + T],
                        in_=slab[:, lo:hi, :],
                    )

                if DBG in ("pure", "mm1"):
                    for g in range(nb // WG):
                        emit_write(g)
                    return

                u_tiles = {}

                def emit_pack(g):
                    """4 rank-16 fill matmuls packed into distinct PE row
                    groups -- they execute concurrently (blocks 2g, 2g+1)."""
                    for m in (2 * g, 2 * g + 1):
                        u_tiles[m] = u_psum.tile([P, cb, T], F32, name="u_ps")
                    for i in range(4):
                        m, c = 2 * g + i // cb, i % cb
                        nc.tensor.matmul(
                            u_tiles[m][:, c, :],
                            bp4[32 * i : 32 * i + RANK, g, :],
                            h1_sb[32 * i : 32 * i + RANK, :],
                            start=True,
                            stop=True,
                            tile_position=(32 * i, 0),
                        )

                emit_pack(0)
                written = 0
                for n in range(nb):
                    if n % 2 == 1 and n + 2 < nb:
                        emit_pack((n + 1) // 2)
                    u_ps = u_tiles.pop(n)
                    kq = kpool.tile([P, cb, T], F16)
                    round_eng.tensor_scalar(
                        kq[:],
                        u_ps[:],
                        MAGIC,
                        MAGIC,
                        mybir.AluOpType.add,
                        mybir.AluOpType.subtract,
                    )
                    for c in range(cb):
                        nc.tensor.matmul(
                            u_ps[:, c, :],
                            nident[:],
                            kq[:, c, :],
                            start=False,
                            stop=True,
                            skip_group_check=True,
                        )
                    s = spool.tile([P, cb, T], F16)
                    nc.scalar.activation(
                        out=s[:],
                        in_=u_ps[:],
                        func=mybir.ActivationFunctionType.Sin,
                        scale=SCALE_2PI,
                    )
                    if DBG == "echo":
                        pass
                    elif DBG == "sin":
                        nc.vector.tensor_copy(slab[:, n * cb : (n + 1) * cb, :], s[:])
                    else:
                        add_eng = nc.gpsimd if ADD_ENG == "gpsimd" else nc.vector
                        add_eng.tensor_tensor(
                            slab[:, n * cb : (n + 1) * cb, :],
                            slab[:, n * cb : (n + 1) * cb, :],
                            s[:],
                            mybir.AluOpType.add,
                        )
                    # write group g once its last add is >= WG blocks behind
                    if (n + 1) % WG == 0 and (n + 1) // WG >= 2:
                        emit_write(written)
                        written += 1
                for g in range(written, nb // WG):
                    emit_write(g)

            prev = None
            for st, (t0, T) in enumerate(layout):
                slab = slab_pool.tile([P, KC, T], F16)
                if HOST_T:
                    nc.sync.dma_start(out=slab[:], in_=x_t[:, :, t0 : t0 + T])
                else:
                    nc.sync.dma_start_transpose(out=slab[:], in_=x_d[t0 : t0 + T, :])
                if prev is not None:
                    emit_tail(prev)
                h1_4 = None
                if DBG != "pure":
                    h1_ps = h1_psum.tile([RANK, T], F32, name="h1_ps")
                    for k in range(KC):
                        nc.tensor.matmul(
                            h1_ps[:],
                            a_sb[:, k, :],
                            slab[:, k, :],
                            start=(k == 0),
                            stop=(k == KC - 1),
                        )
                    h1_sb = h1pool.tile([RANK, T], F32R, name="h1_sb")
                    nc.vector.tensor_copy(h1_sb[:], h1_ps[:])
                    # replicate h1T into all 4 partition groups for the
                    # row-group-packed fill matmuls
                    h1_4ps = h1_psum.tile([P, T], F32, name="h1_4ps")
                    nc.tensor.matmul(
                        h1_4ps[:], rep_sb[:], h1_sb[:], start=True, stop=True
                    )
                    h1_4 = h1pool.tile([P, T], F32R, name="h1_4")
                    nc.vector.tensor_copy(h1_4[:], h1_4ps[:])
                prev = (slab, h1_4, t0, T)

            emit_tail(prev)

    nc.compile()
    return nc


_NC_CACHE: dict[int, object] = {}


def _get_nc(rows: int = ROWS):
    nc = _NC_CACHE.get(rows)
    if nc is None:
        nc = build_nc(rows)
        _NC_CACHE[rows] = nc
    return nc


def _prep_weights(A_int8, B_int8, scale_A, scale_B):
    a2 = np.ascontiguousarray((A_int8.astype(np.float32) * 2.0).astype(np.float16))
    bp = np.ascontiguousarray(
        scale_A.astype(np.float32)[:, None]
        * B_int8.astype(np.float32)
        * scale_B.astype(np.float32)[None, :]
        * np.float32(1.0 / (np.pi * np.pi))
    )
    rep = np.zeros((RANK, P), dtype=np.float32)
    for i in range(4):
        rep[np.arange(RANK), 32 * i + np.arange(RANK)] = 1.0
    return a2, bp, rep


def _prep_in_maps(x, A_int8, B_int8, scale_A, scale_B, rows=ROWS, n_cores=N_CORES):
    xf = x.reshape(-1, HIDDEN)
    a2, bp, rep = _prep_weights(A_int8, B_int8, scale_A, scale_B)
    def shard(i):
        xh = (xf[i * rows : (i + 1) * rows] * np.float32(0.5)).astype(np.float16)
        if HOST_T:
            xh = xh.T
        return np.ascontiguousarray(xh)

    return [
        {"x": shard(i), "A": a2, "Bp": bp, "REP": rep}
        for i in range(n_cores)
    ]


def _postprocess(results, orig_shape):
    parts = [
        (np.ascontiguousarray(r["out"].T).astype(np.float32) * np.float32(2.0))
        for r in results
    ]
    return np.concatenate(parts, axis=0).reshape(orig_shape)


def kernel(x, A_int8, B_int8, scale_A, scale_B):
    x = np.asarray(x)
    orig_shape = x.shape
    in_maps = _prep_in_maps(
        np.ascontiguousarray(x.reshape(TOTAL_ROWS, HIDDEN)),
        np.asarray(A_int8),
        np.asarray(B_int8),
        np.asarray(scale_A),
        np.asarray(scale_B),
    )
    nc = _get_nc(ROWS)
    res = run_bass_kernel_spmd(nc, in_maps, core_ids=list(range(N_CORES)))
    return _postprocess(res.results, orig_shape)


# revision 37
# speedup vs baseline: 1.4932x; 1.1035x over previous
"""PiLoraLayer TRN2 kernel: y = x + (alpha/r) * sin((2/pi) * (x @ A) @ B).

x: [4, 4096, 4096] f32; A = A_int8 * scale_A (per-col), B = B_int8 * scale_B
(per-col); rank 16 bottleneck.

Strategy (data-parallel over 8 NeuronCores, fully transposed dataflow):
- Host: xh = (x/2) as fp16 (residual+matmul source), A2 = 2*A_int8 as fp16,
  Bp = scale_A[:,None] * B_q * scale_B[None,:] / pi^2 (f32). Then
  u = (xh @ A2) @ Bp = arg/(2*pi) and y = 2*(xh + sin(2*pi*frac(u))).
- Shard x's 16384 token rows into 8 x [2048, 4096] shards, one per core.
- Device (per core), per super-tile of T tokens, ALL in transposed space:
    - ONE dma_start_transpose loads xT slab [128h, KC, T] (xbar HW transpose)
    - mm1: h1T[16, T] = sum_k A2_k.T @ xT_k (PSUM accumulate, 32 chunks)
    - mm2 (transposed): uT chunk [128h, T] = Bp_chunk.T @ h1T -- Bp slices
      are natural stationaries, no transpose of anything needed (f32r for
      precision; fp16 Bp would add ~5e-3 rel err).
    - Range reduction per 1024-elem block: kq = (u + 1.5*2^23) - 1.5*2^23
      (one DVE tensor_scalar, RNE-to-integer, fp16 exact for |k|<=2048);
      PE accumulates -kq via fp16 negative-identity matmul -> frac in PSUM;
      ACT: s = sin(2*pi*frac) -> fp16.
    - DVE fp16 TT add (2x mode): slab += s  (slab holds xh.T = x.T/2)
    - DMA slab chunk out as y.T/2; host returns 2 * y_t.T.
- No PE transposes (HAM-blind + PSUM copies), no ACT copies, single x read,
  fp16 I/O: 3x less DMA bytes and ~2x less engine work than the v1 kernel.
"""

import os
import sys

sys.path.insert(0, "/opt/trn_rl_repo")

import numpy as np

HOST_T = bool(os.environ.get("HOST_T"))  # debug: host-side x transpose
DBG = os.environ.get("DBG", "")  # debug: "echo" = skip add, "sin" = only sin part
YW = os.environ.get("YW", "gpsimd")  # y-write engine: gpsimd (SWDGE) | act (HWDGE)
ROUND_ENG = os.environ.get("ROUND", "vector")  # round engine (GPSIMD can't read PSUM)
ADD_ENG = os.environ.get("ADD", "vector")  # residual-add engine: vector | gpsimd

import concourse.bacc as bacc
import concourse.tile as tile
from concourse import mybir
from concourse.bass import ts
from concourse.bass_utils import run_bass_kernel_spmd

P = 128
HIDDEN = 4096
RANK = 16
N_CORES = 8
TOTAL_ROWS = 4 * 4096
ROWS = TOTAL_ROWS // N_CORES  # 2048 per core
SUPER = 512  # tokens per steady-state super-tile
KC = HIDDEN // P  # 32 hidden chunks
BLK = 1024  # tail block free elems (2 PSUM banks)
MAGIC = 12582912.0  # 1.5 * 2^23: f32 add/sub rounds to nearest integer
SCALE_2PI = 6.283185  # slightly < 2*pi so the LUT arg stays inside [-pi, pi]

F32 = mybir.dt.float32
F32R = mybir.dt.float32r
F16 = mybir.dt.float16


def build_nc(rows: int = ROWS):
    """Build the per-core Bass program for a [rows, 4096] token shard."""
    nc = bacc.Bacc(
        "TRN2",
        target_bir_lowering=False,
        debug=False,
        enable_asserts=False,
        num_devices=N_CORES,
    )
    if HOST_T:
        x_d = nc.dram_tensor("x", [HIDDEN, rows], F16, kind="ExternalInput").ap()
        x_t = x_d.rearrange("(k p) t -> p k t", p=P)
    else:
        x_d = nc.dram_tensor("x", [rows, HIDDEN], F16, kind="ExternalInput").ap()
    a_d = nc.dram_tensor("A", [HIDDEN, RANK], F16, kind="ExternalInput").ap()
    bp_d = nc.dram_tensor("Bp", [RANK, HIDDEN], F32, kind="ExternalInput").ap()
    rep_d = nc.dram_tensor("REP", [RANK, P], F32, kind="ExternalInput").ap()
    y_d = nc.dram_tensor("out", [HIDDEN, rows], F16, kind="ExternalOutput").ap()
    y_r = y_d.rearrange("(k p) t -> p k t", p=P)  # [128, KC, rows]

    # uniform super-tiles: PSUM matmul outputs must stay bank-aligned, which
    # holds only for T=512 (chunk writes at 0/512 f32 offsets)
    assert rows % SUPER == 0
    layout = [(i * SUPER, SUPER) for i in range(rows // SUPER)]

    with tile.TileContext(nc) as tc:
        with (
            tc.tile_pool(name="singles", bufs=1) as singles,
            tc.tile_pool(name="slabp", bufs=3) as slab_pool,
            tc.tile_pool(name="kp", bufs=4) as kpool,
            tc.tile_pool(name="sp", bufs=4) as spool,
            tc.tile_pool(name="h1sb", bufs=2) as h1pool,
            tc.tile_pool(name="h1p", bufs=1, space="PSUM") as h1_psum,
            tc.tile_pool(name="up", bufs=3, space="PSUM") as u_psum,
        ):
            nident = singles.tile([P, P], F16)
            nc.gpsimd.memset(nident[:], 0.0)
            nc.gpsimd.affine_select(
                out=nident[:],
                in_=nident[:],
                compare_op=mybir.AluOpType.not_equal,
                fill=-1.0,
                base=0,
                pattern=[[-1, P]],
                channel_multiplier=1,
            )
            a_sb = singles.tile([P, KC, RANK], F16)
            nc.sync.dma_start(
                out=a_sb[:], in_=a_d.rearrange("(k p) r -> p k r", p=P)
            )
            # NOTE: do NOT dma with .bitcast(F32R) in a program that also uses
            # dma_start_transpose -- the f32r-tagged DMA descriptor poisons the
            # xbar path and the transposed f16 data comes back f32r-rounded
            # (even 16-bit lanes quantized). Load plain f32, then produce the
            # f32r copy on-chip with DVE (a legal f32r producer for the PE).
            # bp4[32i : 32i+16, g, :] = Bp[:, (4g+i)*128 : (4g+i+1)*128] --
            # the 4 partition groups let 4 rank-16 fill matmuls run
            # concurrently in distinct PE row groups (tile_position).
            bp4_f32 = singles.tile([P, KC // 4, P], F32)
            bp_r4 = bp_d.rearrange("r (g i c) -> r g i c", g=KC // 4, i=4)
            for i in range(4):
                nc.sync.dma_start(
                    out=bp4_f32[32 * i : 32 * i + RANK, :, :],
                    in_=bp_r4[:, :, i, :],
                )
            bp4 = singles.tile([P, KC // 4, P], F32R)
            for i in range(4):
                nc.vector.tensor_copy(
                    bp4[32 * i : 32 * i + RANK, :, :],
                    bp4_f32[32 * i : 32 * i + RANK, :, :],
                )
            # REP[r, 32i+r] = 1: one PE matmul replicates h1T into all four
            # 32-partition groups so packed fills see matching partition bases
            rep_f32 = singles.tile([RANK, P], F32)
            nc.sync.dma_start(out=rep_f32[:], in_=rep_d[:, :])
            rep_sb = singles.tile([RANK, P], F32R)
            nc.vector.tensor_copy(rep_sb[:], rep_f32[:])

            round_eng = nc.gpsimd if ROUND_ENG == "gpsimd" else nc.vector
            WG = 4  # tail blocks per y-write DMA

            def emit_tail(state):
                """Tail for a finished super-tile, software-pipelined:
                FILL runs one block ahead of SUB so the PE never stalls on
                the round; y-writes batch WG blocks and trail the adds."""
                slab, h1_sb, t0, T = state
                cb = BLK // T  # hidden chunks per block
                nb = KC // cb  # blocks per super-tile
                ydma = nc.scalar.dma_start if YW == "act" else nc.gpsimd.dma_start

                def emit_write(g):
                    lo, hi = g * WG * cb, (g + 1) * WG * cb
                    ydma(
                        out=y_r[:, lo:hi, t0 : t0 + T],
                        in_=slab[:, lo:hi, :],
                    )

                if DBG in ("pure", "mm1"):
                    for g in range(nb // WG):
                        emit_write(g)
                    return

                u_tiles = {}

                def emit_pack(g):
                    """4 rank-16 fill matmuls packed into distinct PE row
                    groups -- they execute concurrently (blocks 2g, 2g+1)."""
                    for m in (2 * g, 2 * g + 1):
                        u_tiles[m] = u_psum.tile([P, cb, T], F32, name="u_ps")
                    for i in range(4):
                        m, c = 2 * g + i // cb, i % cb
                        nc.tensor.matmul(
                            u_tiles[m][:, c, :],
                            bp4[32 * i : 32 * i + RANK, g, :],
                            h1_sb[32 * i : 32 * i + RANK, :],
                            start=True,
                            stop=True,
                            tile_position=(32 * i, 0),
                        )

                def emit_add(m, s):
                    """Residual add for block m (deferred 2 blocks so the DVE
                    queue never stalls on sin), then the batched y-write."""
                    if DBG == "echo":
                        pass
                    elif DBG == "sin":
                        nc.vector.tensor_copy(slab[:, m * cb : (m + 1) * cb, :], s[:])
                    else:
                        add_eng = nc.gpsimd if ADD_ENG == "gpsimd" else nc.vector
                        add_eng.tensor_tensor(
                            slab[:, m * cb : (m + 1) * cb, :],
                            slab[:, m * cb : (m + 1) * cb, :],
                            s[:],
                            mybir.AluOpType.add,
                        )
                    if (m + 1) % WG == 0:
                        emit_write((m + 1) // WG - 1)

                emit_pack(0)
                s_tiles = {}
                for n in range(nb):
                    if n % 2 == 1 and n + 2 < nb:
                        emit_pack((n + 1) // 2)
                    u_ps = u_tiles.pop(n)
                    kq = kpool.tile([P, cb, T], F16)
                    round_eng.tensor_scalar(
                        kq[:],
                        u_ps[:],
                        MAGIC,
                        MAGIC,
                        mybir.AluOpType.add,
                        mybir.AluOpType.subtract,
                    )
                    for c in range(cb):
                        nc.tensor.matmul(
                            u_ps[:, c, :],
                            nident[:],
                            kq[:, c, :],
                            start=False,
                            stop=True,
                            skip_group_check=True,
                        )
                    s = spool.tile([P, cb, T], F16)
                    nc.scalar.activation(
                        out=s[:],
                        in_=u_ps[:],
                        func=mybir.ActivationFunctionType.Sin,
                        scale=SCALE_2PI,
                    )
                    s_tiles[n] = s
                    if n >= 2:
                        emit_add(n - 2, s_tiles.pop(n - 2))
                for m in (nb - 2, nb - 1):
                    emit_add(m, s_tiles.pop(m))

            prev = None
            for st, (t0, T) in enumerate(layout):
                slab = slab_pool.tile([P, KC, T], F16)
                if HOST_T:
                    nc.sync.dma_start(out=slab[:], in_=x_t[:, :, t0 : t0 + T])
                else:
                    nc.sync.dma_start_transpose(out=slab[:], in_=x_d[t0 : t0 + T, :])
                if prev is not None:
                    emit_tail(prev)
                h1_4 = None
                if DBG != "pure":
                    h1_ps = h1_psum.tile([RANK, T], F32, name="h1_ps")
                    for k in range(KC):
                        nc.tensor.matmul(
                            h1_ps[:],
                            a_sb[:, k, :],
                            slab[:, k, :],
                            start=(k == 0),
                            stop=(k == KC - 1),
                        )
                    h1_sb = h1pool.tile([RANK, T], F32R, name="h1_sb")
                    nc.vector.tensor_copy(h1_sb[:], h1_ps[:])
                    # replicate h1T into all 4 partition groups for the
                    # row-group-packed fill matmuls
                    h1_4ps = h1_psum.tile([P, T], F32, name="h1_4ps")
                    nc.tensor.matmul(
                        h1_4ps[:], rep_sb[:], h1_sb[:], start=True, stop=True
                    )
                    h1_4 = h1pool.tile([P, T], F32R, name="h1_4")
                    nc.vector.tensor_copy(h1_4[:], h1_4ps[:])
                prev = (slab, h1_4, t0, T)

            emit_tail(prev)

    nc.compile()
    return nc


_NC_CACHE: dict[int, object] = {}


def _get_nc(rows: int = ROWS):
    nc = _NC_CACHE.get(rows)
    if nc is None:
        nc = build_nc(rows)
        _NC_CACHE[rows] = nc
    return nc


def _prep_weights(A_int8, B_int8, scale_A, scale_B):
    a2 = np.ascontiguousarray((A_int8.astype(np.float32) * 2.0).astype(np.float16))
    bp = np.ascontiguousarray(
        scale_A.astype(np.float32)[:, None]
        * B_int8.astype(np.float32)
        * scale_B.astype(np.float32)[None, :]
        * np.float32(1.0 / (np.pi * np.pi))
    )
    rep = np.zeros((RANK, P), dtype=np.float32)
    for i in range(4):
        rep[np.arange(RANK), 32 * i + np.arange(RANK)] = 1.0
    return a2, bp, rep


def _prep_in_maps(x, A_int8, B_int8, scale_A, scale_B, rows=ROWS, n_cores=N_CORES):
    xf = x.reshape(-1, HIDDEN)
    a2, bp, rep = _prep_weights(A_int8, B_int8, scale_A, scale_B)
    def shard(i):
        xh = (xf[i * rows : (i + 1) * rows] * np.float32(0.5)).astype(np.float16)
        if HOST_T:
            xh = xh.T
        return np.ascontiguousarray(xh)

    return [
        {"x": shard(i), "A": a2, "Bp": bp, "REP": rep}
        for i in range(n_cores)
    ]


def _postprocess(results, orig_shape):
    parts = [
        (np.ascontiguousarray(r["out"].T).astype(np.float32) * np.float32(2.0))
        for r in results
    ]
    return np.concatenate(parts, axis=0).reshape(orig_shape)


def kernel(x, A_int8, B_int8, scale_A, scale_B):
    x = np.asarray(x)
    orig_shape = x.shape
    in_maps = _prep_in_maps(
        np.ascontiguousarray(x.reshape(TOTAL_ROWS, HIDDEN)),
        np.asarray(A_int8),
        np.asarray(B_int8),
        np.asarray(scale_A),
        np.asarray(scale_B),
    )
    nc = _get_nc(ROWS)
    res = run_bass_kernel_spmd(nc, in_maps, core_ids=list(range(N_CORES)))
    return _postprocess(res.results, orig_shape)
